# revision 5
# baseline (speedup 1.0000x reference)
"""Non-local (dot-product attention) block kernel for Trainium2, 8 cores.

Math (per sample, fp32 reference):
    t = theta_w @ xf + theta_b          (D, N)   "theta", D=128, N=3072
    p = (phi_w @ xf + phi_b) / N        (D, N)   "phi" (1/N folded in)
    g = g_w @ xf + g_b                  (D, N)
    M[e,d]   = sum_m p[e,m] g[d,m]               (attention collapsed by
    y[n,d]   = sum_e t[e,n] M[e,d]                matmul associativity --
    z[c,n]   = sum_d w'[c,d] y[n,d] + b'[c] + x   the NxN matrix never exists)
with BN folded on host: w' = diag(gamma/sqrt(var+eps)) @ w_w,
b' = (w_b - mean)*inv + beta.

Sharding: pure data-parallel over batch B=8, one sample per NeuronCore,
weights replicated, no collectives.

Precision: matmul inputs are bf16 (weights converted on host, x shipped as
bf16), accumulation fp32 in PSUM; biases and the residual are applied in
fp32. x is also shipped... no -- x arrives only as bf16; the residual add
uses the bf16 x (error ~1e-4 of output scale). Output is fp32.

On-chip layouts per core (partition dim first):
    X0/X1   (128, 3072) bf16  xf row-halves, c on partitions
    t_sb    (128, 3072) bf16  theta in (D, N)
    pg_sb   (128, 24*256) bf16  phi|g per 128-pixel chunk, n on partitions
    m2_sb   (128, 128)  bf16  m2[d,e] = M[e,d]
    w2_sb   (128, 256)  bf16  w2[e,c] = sum_d m2[d,e] w'[c,d]
    z = w2.T @ t + b' + x, fp32, streamed out in (128, 512) chunks.
"""

import numpy as np

B, C, HH, WW = 8, 256, 96, 32
N = HH * WW          # 3072
D = 128              # inter_channels
BN_EPS = 1e-5
NT = N // 128        # 24 pixel chunks
NF = N // 512        # 6 wide chunks
N_CORES = 8

_NC = None


def _build_nc():
    from contextlib import ExitStack

    import concourse.bacc as bacc
    import concourse.tile as tile
    from concourse import mybir

    f32 = mybir.dt.float32
    bf16 = mybir.dt.bfloat16
    AF = mybir.ActivationFunctionType
    ALU = mybir.AluOpType

    nc = bacc.Bacc(
        "TRN2",
        target_bir_lowering=False,
        debug=False,
        num_devices=N_CORES,
    )

    x = nc.dram_tensor("x", [C, N], bf16, kind="ExternalInput").ap()
    thw = nc.dram_tensor("thw", [C, D], bf16, kind="ExternalInput").ap()
    pgw = nc.dram_tensor("pgw", [C, 2 * D], bf16, kind="ExternalInput").ap()
    wwt = nc.dram_tensor("wwt", [D, C], bf16, kind="ExternalInput").ap()
    aux = nc.dram_tensor("aux", [128, 260], f32, kind="ExternalInput").ap()
    out = nc.dram_tensor("out", [C, N], f32, kind="ExternalOutput").ap()

    with tile.TileContext(nc) as tc, ExitStack() as ctx:
        const = ctx.enter_context(tc.tile_pool(name="const", bufs=1))
        zpool = ctx.enter_context(tc.tile_pool(name="zpool", bufs=3))
        ps_t = ctx.enter_context(tc.tile_pool(name="ps_t", bufs=2, space="PSUM"))
        ps_pg = ctx.enter_context(tc.tile_pool(name="ps_pg", bufs=3, space="PSUM"))
        ps_m2 = ctx.enter_context(tc.tile_pool(name="ps_m2", bufs=1, space="PSUM"))
        ps_z = ps_t

        X0 = const.tile([128, N], bf16)
        X1 = const.tile([128, N], bf16)
        t_sb = const.tile([128, N], bf16)
        pg_sb = const.tile([128, NT * 256], bf16)
        m2_sb = const.tile([128, 128], bf16)
        w2_sb = const.tile([128, 256], bf16)
        thetaT = const.tile([128, 256], bf16)
        pgW = const.tile([128, 512], bf16)
        wT = const.tile([128, 256], bf16)
        aux_sb = const.tile([128, 260], f32)

        # weight / bias loads
        nc.sync.dma_start(out=aux_sb, in_=aux)
        nc.sync.dma_start(out=thetaT[:, 0:128], in_=thw[0:128, :])
        nc.sync.dma_start(out=thetaT[:, 128:256], in_=thw[128:256, :])
        nc.sync.dma_start(out=pgW[:, 0:256], in_=pgw[0:128, :])
        nc.sync.dma_start(out=pgW[:, 256:512], in_=pgw[128:256, :])
        nc.sync.dma_start(out=wT, in_=wwt)

        theta_b = aux_sb[:, 0:1]
        b_out = [aux_sb[:, 1:2], aux_sb[:, 2:3]]
        pg_bias = aux_sb[:, 4:260]

        # x loads, chunked along n so compute can start early
        XCH = 1024
        for cch in range(N // XCH):
            sl = slice(cch * XCH, (cch + 1) * XCH)
            nc.sync.dma_start(out=X0[:, sl], in_=x[0:128, sl])
            nc.sync.dma_start(out=X1[:, sl], in_=x[128:256, sl])

        # phi|g projection, (N, D) layout: chunk nt holds pixels nt*128..+127
        # on partitions, [phi | g] along free dim. Bias added during the
        # PSUM->SBUF copy via a pre-broadcast bias tile.
        for nt in range(NT):
            nsl = slice(nt * 128, (nt + 1) * 128)
            pp = ps_pg.tile([128, 256], f32, tag="pg")
            nc.tensor.matmul(
                pp, lhsT=X0[:, nsl], rhs=pgW[:, 0:256], start=True, stop=False
            )
            nc.tensor.matmul(
                pp, lhsT=X1[:, nsl], rhs=pgW[:, 256:512], start=False, stop=True
            )
            nc.vector.tensor_add(pg_sb[:, nt * 256 : (nt + 1) * 256], pp, pg_bias)

        # theta projection, (D, N) layout; bias via ACT per-partition bias
        for f in range(NF):
            fsl = slice(f * 512, (f + 1) * 512)
            pt = ps_t.tile([128, 512], f32, tag="t")
            nc.tensor.matmul(
                pt, lhsT=thetaT[:, 0:128], rhs=X0[:, fsl], start=True, stop=False
            )
            nc.tensor.matmul(
                pt, lhsT=thetaT[:, 128:256], rhs=X1[:, fsl], start=False, stop=True
            )
            nc.scalar.activation(
                out=t_sb[:, fsl], in_=pt, func=AF.Identity, bias=theta_b, scale=1.0
            )

        # m2[d,e] = sum_m g[m,d] p[m,e], accumulated over all 24 pixel chunks.
        # rhs is the full [p|g] chunk (256 wide); the g.T@g half of the
        # output is discarded.
        pm = ps_m2.tile([128, 256], f32, tag="m2")
        for nt in range(NT):
            nc.tensor.matmul(
                pm,
                lhsT=pg_sb[:, nt * 256 + 128 : (nt + 1) * 256],
                rhs=pg_sb[:, nt * 256 : (nt + 1) * 256],
                start=(nt == 0),
                stop=(nt == NT - 1),
            )
        nc.scalar.copy(out=m2_sb, in_=pm[:, 0:128])

        # w2[e,c] = sum_d m2[d,e] w'[c,d]
        pw = ps_m2.tile([128, 256], f32, tag="m2")
        nc.tensor.matmul(pw, lhsT=m2_sb, rhs=wT, start=True, stop=True)
        nc.scalar.copy(out=w2_sb, in_=pw)

        # z[c,n] = sum_e w2[e,c] t[e,n] + b'[c] + x[c,n], streamed to DRAM
        for cc in range(2):
            csl = slice(cc * 128, (cc + 1) * 128)
            xsrc = X0 if cc == 0 else X1
            for f in range(NF):
                fsl = slice(f * 512, (f + 1) * 512)
                pz = ps_z.tile([128, 512], f32, tag="t")
                nc.tensor.matmul(
                    pz, lhsT=w2_sb[:, csl], rhs=t_sb[:, fsl], start=True, stop=True
                )
                z_sb = zpool.tile([128, 512], f32, tag="z_sb")
                nc.vector.scalar_tensor_tensor(
                    out=z_sb,
                    in0=pz,
                    scalar=b_out[cc],
                    in1=xsrc[:, fsl],
                    op0=ALU.add,
                    op1=ALU.add,
                )
                nc.sync.dma_start(out=out[csl, fsl], in_=z_sb)

    nc.compile()
    return nc


def _get_nc():
    global _NC
    if _NC is None:
        _NC = _build_nc()
    return _NC


# test.py reads this after a traced run to get exec_time_ns
last_results = None


def kernel(**inputs):
    import ml_dtypes

    from concourse.bass_utils import run_bass_kernel_spmd

    global last_results

    bf16 = ml_dtypes.bfloat16

    x = np.asarray(inputs["x"], dtype=np.float32)
    g_w = np.asarray(inputs["g_w"], np.float32)
    g_b = np.asarray(inputs["g_b"], np.float32)
    theta_w = np.asarray(inputs["theta_w"], np.float32)
    theta_b = np.asarray(inputs["theta_b"], np.float32)
    phi_w = np.asarray(inputs["phi_w"], np.float32)
    phi_b = np.asarray(inputs["phi_b"], np.float32)
    w_w = np.asarray(inputs["w_w"], np.float32)
    w_b = np.asarray(inputs["w_b"], np.float32)
    bn_gamma = np.asarray(inputs["bn_gamma"], np.float32)
    bn_beta = np.asarray(inputs["bn_beta"], np.float32)
    bn_mean = np.asarray(inputs["bn_mean"], np.float32)
    bn_var = np.asarray(inputs["bn_var"], np.float32)

    inv = bn_gamma / np.sqrt(bn_var + BN_EPS)
    thw = np.ascontiguousarray(theta_w.T).astype(bf16)                # (C, D)
    pgw = np.ascontiguousarray(
        np.concatenate([phi_w.T / N, g_w.T], axis=1)                  # (C, 2D)
    ).astype(bf16)
    wwt = np.ascontiguousarray((w_w * inv[:, None]).T).astype(bf16)   # (D, C)
    b_out = (w_b - bn_mean) * inv + bn_beta                           # (C,)

    aux = np.zeros((128, 260), np.float32)
    aux[:, 0] = theta_b
    aux[:, 1] = b_out[:128]
    aux[:, 2] = b_out[128:]
    aux[:, 4:260] = np.concatenate([phi_b / N, g_b])[None, :]

    xf = x.reshape(B, C, N).astype(bf16)
    shared = {"thw": thw, "pgw": pgw, "wwt": wwt, "aux": aux}
    in_maps = [dict(shared, x=np.ascontiguousarray(xf[b])) for b in range(B)]

    nc = _get_nc()
    res = run_bass_kernel_spmd(nc, in_maps, list(range(N_CORES)))
    last_results = res

    z = np.stack([res.results[b]["out"] for b in range(B)])
    return z.reshape(B, C, HH, WW).astype(np.float32)


# revision 8
# speedup vs baseline: 1.0708x; 1.0708x over previous
"""Non-local (dot-product attention) block kernel for Trainium2, 8 cores.

Reference math (per sample):
    t = theta_w @ xf + theta_b           (D, N)
    p = (phi_w @ xf + phi_b) / N         (D, N)
    g = g_w @ xf + g_b                   (D, N)
    f = t.T p / 1  (NxN attention);  y = f g.T;  z = BN(w_w y) + x

Algebraic collapse (matmul associativity, BN folded on host):
    M[e,d]  = sum_m p[e,m] g[d,m]                      (D x D)
    V[c,e]  = sum_d w'[c,d] M[e,d]      w' = diag(inv) w_w
    U[c,ci] = sum_e V[c,e] theta_w[e,ci]               (C x C)
    b_z[c]  = sum_e V[c,e] theta_b[e] + b'[c]
    z       = U @ xf + b_z 1^T + x
so the N x N attention matrix, y, and even the theta projection never
exist -- per sample just two passes over x (proj p|g, final U @ x) plus
tiny D x D / C x C matmuls in between.

Sharding: data-parallel over batch B=8, one sample per NeuronCore, no
collectives. Matmul inputs bf16 (fp32 PSUM accumulation), biases and
residual applied in fp32; output fp32.

Per-core layout (partition dim first):
    X[k][j]  6 x (128, 1024) bf16   xf, k = channel half, j = pixel region
    pg_sb    (128, 24*256)   bf16   [phi|g] per 128-pixel chunk, n on parts
    m2_sb    (128, 128) bf16   m2[d,e] = M[e,d]
    w2_sb    (128, 256) bf16   w2[e,c] = V[c,e]
    ut_sb    (128, 512) bf16   ut[ci, c] = U[c,ci], ci-chunk major
    z: per (cc, j): psum (128,1024) = U @ x (K=2 chunks), then one DVE
       scalar_tensor_tensor (+b_z, +x) -> fp32, one 512KB DMA out.
"""

import numpy as np

B, C, HH, WW = 8, 256, 96, 32
N = HH * WW          # 3072
D = 128              # inter_channels
BN_EPS = 1e-5
NT = N // 128        # 24 pixel chunks
NR = N // 1024       # 3 pixel regions
N_CORES = 8

_NC = None


def _build_nc():
    from contextlib import ExitStack

    import concourse.bass as bass
    import concourse.bacc as bacc
    import concourse.tile as tile
    from concourse import mybir

    f32 = mybir.dt.float32
    bf16 = mybir.dt.bfloat16
    AF = mybir.ActivationFunctionType
    ALU = mybir.AluOpType

    nc = bacc.Bacc(
        "TRN2",
        target_bir_lowering=False,
        debug=False,
        num_devices=N_CORES,
    )

    x = nc.dram_tensor("x", [C, N], bf16, kind="ExternalInput").ap()
    # thw: [theta_w (D,C) | theta_b (D,1) | pad] as (128, 258)
    thw = nc.dram_tensor("thw", [D, 258], bf16, kind="ExternalInput").ap()
    pgw = nc.dram_tensor("pgw", [C, 2 * D], bf16, kind="ExternalInput").ap()
    wwt = nc.dram_tensor("wwt", [D, C], bf16, kind="ExternalInput").ap()
    # aux fp32: [b'(c0) | b'(c1) | pad | pad | pg_bias(256)]
    aux = nc.dram_tensor("aux", [128, 260], f32, kind="ExternalInput").ap()
    out = nc.dram_tensor("out", [C, N], f32, kind="ExternalOutput").ap()

    with tile.TileContext(nc) as tc, ExitStack() as ctx:
        const = ctx.enter_context(tc.tile_pool(name="const", bufs=1))
        zpool = ctx.enter_context(tc.tile_pool(name="zpool", bufs=3))
        ps_mm = ctx.enter_context(tc.tile_pool(name="ps_mm", bufs=3, space="PSUM"))
        ps_sm = ctx.enter_context(tc.tile_pool(name="ps_sm", bufs=2, space="PSUM"))

        X = [
            [
                const.tile([128, 1024], bf16, tag=f"x{k}{j}", name=f"x{k}{j}")
                for j in range(NR)
            ]
            for k in range(2)
        ]
        pg_sb = const.tile([128, NT * 256], bf16)
        m2_sb = const.tile([128, 128], bf16)
        w2_sb = const.tile([128, 256], bf16)
        ut_sb = const.tile([128, 512], bf16)
        bz_sb = const.tile([128, 2], f32)
        thw_sb = const.tile([128, 258], bf16)
        pgW = const.tile([128, 512], bf16)
        wT = const.tile([128, 256], bf16)
        aux_sb = const.tile([128, 260], f32)

        # weight / bias loads first (small), then x regions
        nc.sync.dma_start(out=aux_sb, in_=aux)
        nc.sync.dma_start(out=pgW[:, 0:256], in_=pgw[0:128, :])
        nc.sync.dma_start(out=pgW[:, 256:512], in_=pgw[128:256, :])
        nc.sync.dma_start(out=thw_sb, in_=thw)
        nc.sync.dma_start(out=wT, in_=wwt)
        for j in range(NR):
            jsl = slice(j * 1024, (j + 1) * 1024)
            nc.sync.dma_start(out=X[0][j], in_=x[0:128, jsl])
            nc.sync.dma_start(out=X[1][j], in_=x[128:256, jsl])

        b_out = [aux_sb[:, 0:1], aux_sb[:, 1:2]]
        _pgb = aux_sb[:, 4:260]
        pg_bias3 = bass.AP(
            tensor=_pgb.tensor, offset=_pgb.offset,
            ap=[list(_pgb.ap[0]), [0, 4], list(_pgb.ap[1])],
        )

        # m2[d,e] = sum_m g[m,d] p[m,e] accumulates across the whole pg phase
        pm = ps_sm.tile([128, 128], f32, tag="sm")

        # pg projection in (N, D)-chunk layout + interleaved m2 accumulation.
        # Each psum tile holds 4 pixel chunks (2 banks); one DVE add applies
        # the [phi|g] bias via a broadcast view and converts to bf16.
        for grp in range(NT // 4):          # 6 groups of 4 chunks
            j = grp // 2
            pp = ps_mm.tile([128, 1024], f32, tag="mm")
            for i in range(4):
                nt = grp * 4 + i
                loc = (nt % 8) * 128
                nsl = slice(loc, loc + 128)
                psl = slice(i * 256, (i + 1) * 256)
                nc.tensor.matmul(
                    pp[:, psl], lhsT=X[0][j][:, nsl], rhs=pgW[:, 0:256],
                    start=True, stop=False,
                )
                nc.tensor.matmul(
                    pp[:, psl], lhsT=X[1][j][:, nsl], rhs=pgW[:, 256:512],
                    start=False, stop=True,
                )
            gsl = slice(grp * 1024, (grp + 1) * 1024)
            nc.vector.tensor_add(
                pg_sb[:, gsl].rearrange("p (a b) -> p a b", a=4),
                pp.rearrange("p (a b) -> p a b", a=4),
                pg_bias3,
            )
            for i in range(4):
                nt = grp * 4 + i
                nc.tensor.matmul(
                    pm,
                    lhsT=pg_sb[:, nt * 256 + 128 : (nt + 1) * 256],
                    rhs=pg_sb[:, nt * 256 : nt * 256 + 128],
                    start=(nt == 0),
                    stop=(nt == NT - 1),
                )
        nc.scalar.copy(out=m2_sb, in_=pm)

        # w2[e,c] = sum_d m2[d,e] w'[c,d]
        pw = ps_sm.tile([128, 256], f32, tag="sm")
        nc.tensor.matmul(pw, lhsT=m2_sb, rhs=wT, start=True, stop=True)
        nc.scalar.copy(out=w2_sb, in_=pw)

        # ut[ci,c] = sum_e theta_w[e,ci] w2[e,c]   (= U[c,ci])
        for ci in range(2):
            pu = ps_sm.tile([128, 256], f32, tag="sm")
            nc.tensor.matmul(
                pu, lhsT=thw_sb[:, ci * 128 : (ci + 1) * 128], rhs=w2_sb,
                start=True, stop=True,
            )
            nc.scalar.copy(out=ut_sb[:, ci * 256 : (ci + 1) * 256], in_=pu)

        # b_z[c] = sum_e w2[e,c] theta_b[e] + b'[c]
        for cc in range(2):
            pb = ps_sm.tile([128, 1], f32, tag="sm")
            nc.tensor.matmul(
                pb, lhsT=w2_sb[:, cc * 128 : (cc + 1) * 128],
                rhs=thw_sb[:, 256:257], start=True, stop=True,
            )
            nc.scalar.activation(
                out=bz_sb[:, cc : cc + 1], in_=pb, func=AF.Identity,
                bias=b_out[cc], scale=1.0,
            )

        # z[c,n] = sum_ci U[c,ci] x[ci,n] + b_z[c] + x[c,n]
        for j in range(NR):
            for cc in range(2):
                pz = ps_mm.tile([128, 1024], f32, tag="mm")
                for f in range(2):
                    fsl = slice(f * 512, (f + 1) * 512)
                    nc.tensor.matmul(
                        pz[:, fsl],
                        lhsT=ut_sb[:, cc * 128 : (cc + 1) * 128],
                        rhs=X[0][j][:, fsl], start=True, stop=False,
                    )
                    nc.tensor.matmul(
                        pz[:, fsl],
                        lhsT=ut_sb[:, 256 + cc * 128 : 256 + (cc + 1) * 128],
                        rhs=X[1][j][:, fsl], start=False, stop=True,
                    )
                z_sb = zpool.tile([128, 1024], f32, tag="z_sb")
                nc.vector.scalar_tensor_tensor(
                    out=z_sb, in0=pz, scalar=bz_sb[:, cc : cc + 1],
                    in1=X[cc][j], op0=ALU.add, op1=ALU.add,
                )
                nc.sync.dma_start(
                    out=out[cc * 128 : (cc + 1) * 128, j * 1024 : (j + 1) * 1024],
                    in_=z_sb,
                )

    nc.compile()
    return nc


def _get_nc():
    global _NC
    if _NC is None:
        _NC = _build_nc()
    return _NC


# test.py reads this after a traced run to get exec_time_ns
last_results = None


def _prep_inputs(inputs):
    import ml_dtypes

    bf16 = ml_dtypes.bfloat16

    x = np.asarray(inputs["x"], dtype=np.float32)
    theta_w = np.asarray(inputs["theta_w"], np.float32)
    theta_b = np.asarray(inputs["theta_b"], np.float32)
    phi_w = np.asarray(inputs["phi_w"], np.float32)
    phi_b = np.asarray(inputs["phi_b"], np.float32)
    g_w = np.asarray(inputs["g_w"], np.float32)
    g_b = np.asarray(inputs["g_b"], np.float32)
    w_w = np.asarray(inputs["w_w"], np.float32)
    w_b = np.asarray(inputs["w_b"], np.float32)
    bn_gamma = np.asarray(inputs["bn_gamma"], np.float32)
    bn_beta = np.asarray(inputs["bn_beta"], np.float32)
    bn_mean = np.asarray(inputs["bn_mean"], np.float32)
    bn_var = np.asarray(inputs["bn_var"], np.float32)

    inv = bn_gamma / np.sqrt(bn_var + BN_EPS)
    thw = np.zeros((D, 258), np.float32)
    thw[:, :256] = theta_w                                    # (D, C)
    thw[:, 256] = theta_b
    pgw = np.concatenate([phi_w.T / N, g_w.T], axis=1)        # (C, 2D)
    wwt = np.ascontiguousarray((w_w * inv[:, None]).T)        # (D, C)
    b_out = (w_b - bn_mean) * inv + bn_beta                   # (C,)

    aux = np.zeros((128, 260), np.float32)
    aux[:, 0] = b_out[:128]
    aux[:, 1] = b_out[128:]
    aux[:, 4:260] = np.concatenate([phi_b / N, g_b])[None, :]

    xf = x.reshape(B, C, N).astype(bf16)
    shared = {
        "thw": thw.astype(bf16),
        "pgw": np.ascontiguousarray(pgw).astype(bf16),
        "wwt": wwt.astype(bf16),
        "aux": aux,
    }
    return xf, shared


def kernel(**inputs):
    from concourse.bass_utils import run_bass_kernel_spmd

    global last_results

    xf, shared = _prep_inputs(inputs)
    in_maps = [dict(shared, x=np.ascontiguousarray(xf[b])) for b in range(B)]

    nc = _get_nc()
    res = run_bass_kernel_spmd(nc, in_maps, list(range(N_CORES)))
    last_results = res

    z = np.stack([res.results[b]["out"] for b in range(B)])
    return z.reshape(B, C, HH, WW).astype(np.float32)


# revision 9
# speedup vs baseline: 1.0722x; 1.0013x over previous
"""Non-local (dot-product attention) block kernel for Trainium2, 8 cores.

Reference math (per sample):
    t = theta_w @ xf + theta_b           (D, N)
    p = (phi_w @ xf + phi_b) / N         (D, N)
    g = g_w @ xf + g_b                   (D, N)
    f = t.T p  (NxN attention);  y = f g.T;  z = BN(w_w y) + x

Algebraic collapse (matmul associativity, BN folded on host):
    M[e,d]  = sum_m p[e,m] g[d,m]                      (D x D)
    V[c,e]  = sum_d w'[c,d] M[e,d]      w' = diag(inv) w_w
    U[c,ci] = sum_e V[c,e] theta_w[e,ci]               (C x C)
    b_z[c]  = sum_e V[c,e] theta_b[e] + b'[c]
    z       = U @ xf + b_z 1^T + x
so the N x N attention matrix, y, and the theta projection never exist --
per sample just two passes over x (proj p|g, final U @ x) plus tiny
D x D / C x C matmuls in between.

Sharding: data-parallel over batch B=8, one sample per NeuronCore, no
collectives. Matmul inputs bf16 (fp32 PSUM accumulation), biases and
residual applied in fp32; output fp32.

HW notes baked in:
  - All weights/biases ship as ONE byte-packed DMA (each dma_start has a
    ~600ns fixed cost and small rows kill DMA descriptor throughput).
  - x halves are monolithic (128, 3072) bf16 DMAs (6KB descriptors) split
    across the two HWDGE rings (sync + scalar) for parallel issue.
  - Dummy matmuls bridge the small-matmul chain between the pg phase and
    the z phase so the PE HAM clock stays at 2.4 GHz for the z matmuls.
  - Output DMAs alternate rings, (128, 1024) fp32 chunks.
"""

import numpy as np

B, C, HH, WW = 8, 256, 96, 32
N = HH * WW          # 3072
D = 128              # inter_channels
BN_EPS = 1e-5
NT = N // 128        # 24 pixel chunks
NR = N // 1024       # 3 pixel regions
N_CORES = 8

_NC = None


def _build_nc():
    from contextlib import ExitStack

    import concourse.bass as bass
    import concourse.bacc as bacc
    import concourse.tile as tile
    from concourse import mybir

    f32 = mybir.dt.float32
    bf16 = mybir.dt.bfloat16
    AF = mybir.ActivationFunctionType
    ALU = mybir.AluOpType

    nc = bacc.Bacc(
        "TRN2",
        target_bir_lowering=False,
        debug=False,
        num_devices=N_CORES,
    )

    x = nc.dram_tensor("x", [C, N], bf16, kind="ExternalInput").ap()
    # wpk byte-packs, per partition row: aux 260 f32 | pgW 512 bf16 |
    # thw 260 bf16 | wT 256 bf16  => 774 f32 columns total
    wpk = nc.dram_tensor("wpk", [128, 774], f32, kind="ExternalInput").ap()
    out = nc.dram_tensor("out", [C, N], f32, kind="ExternalOutput").ap()

    with tile.TileContext(nc) as tc, ExitStack() as ctx:
        const = ctx.enter_context(tc.tile_pool(name="const", bufs=1))
        zpool = ctx.enter_context(tc.tile_pool(name="zpool", bufs=3))
        ps_mm = ctx.enter_context(tc.tile_pool(name="ps_mm", bufs=3, space="PSUM"))
        ps_sm = ctx.enter_context(tc.tile_pool(name="ps_sm", bufs=2, space="PSUM"))

        X0 = const.tile([128, N], bf16)
        X1 = const.tile([128, N], bf16)
        pg_sb = const.tile([128, NT * 256], bf16)
        m2_sb = const.tile([128, 128], bf16)
        w2_sb = const.tile([128, 256], bf16)
        ut_sb = const.tile([128, 512], bf16)
        bz_sb = const.tile([128, 2], f32)
        wpk_sb = const.tile([128, 774], f32)

        # one packed weight DMA (scalar ring), then x halves on both rings
        nc.scalar.dma_start(out=wpk_sb, in_=wpk)
        nc.sync.dma_start(out=X0, in_=x[0:128, :])
        nc.scalar.dma_start(out=X1, in_=x[128:256, :])

        aux_sb = wpk_sb[:, 0:260]
        pgW = wpk_sb[:, 260:516].bitcast(bf16)     # (128, 512)
        thw_sb = wpk_sb[:, 516:646].bitcast(bf16)  # (128, 260)
        wT = wpk_sb[:, 646:774].bitcast(bf16)      # (128, 256)

        b_out = [aux_sb[:, 0:1], aux_sb[:, 1:2]]
        _pgb = aux_sb[:, 4:260]
        pg_bias3 = bass.AP(
            tensor=_pgb.tensor, offset=_pgb.offset,
            ap=[list(_pgb.ap[0]), [0, 4], list(_pgb.ap[1])],
        )

        # m2[d,e] = sum_m g[m,d] p[m,e] accumulates across the whole pg phase
        pm = ps_sm.tile([128, 128], f32, tag="sm")

        # pg projection in (N, D)-chunk layout + interleaved m2 accumulation.
        # Each psum tile holds 4 pixel chunks (2 banks); one DVE add applies
        # the [phi|g] bias via a broadcast view and converts to bf16.
        for grp in range(NT // 4):          # 6 groups of 4 chunks
            pp = ps_mm.tile([128, 1024], f32, tag="mm")
            for i in range(4):
                nt = grp * 4 + i
                nsl = slice(nt * 128, (nt + 1) * 128)
                psl = slice(i * 256, (i + 1) * 256)
                nc.tensor.matmul(
                    pp[:, psl], lhsT=X0[:, nsl], rhs=pgW[:, 0:256],
                    start=True, stop=False,
                )
                nc.tensor.matmul(
                    pp[:, psl], lhsT=X1[:, nsl], rhs=pgW[:, 256:512],
                    start=False, stop=True,
                )
            gsl = slice(grp * 1024, (grp + 1) * 1024)
            nc.vector.tensor_add(
                pg_sb[:, gsl].rearrange("p (a b) -> p a b", a=4),
                pp.rearrange("p (a b) -> p a b", a=4),
                pg_bias3,
            )
            for i in range(4):
                nt = grp * 4 + i
                nc.tensor.matmul(
                    pm,
                    lhsT=pg_sb[:, nt * 256 + 128 : (nt + 1) * 256],
                    rhs=pg_sb[:, nt * 256 : nt * 256 + 128],
                    start=(nt == 0),
                    stop=(nt == NT - 1),
                )
        nc.scalar.copy(out=m2_sb, in_=pm)

        # dummy matmuls keep the PE HAM activity window busy while the small
        # serial m2 -> w2 -> ut -> bz chain runs, so the z matmuls run warm
        def dummy_mms(k):
            dmy = ps_mm.tile([128, 512], f32, tag="mm", name=f"dmy{k}")
            for _ in range(3):
                nc.tensor.matmul(
                    dmy, lhsT=wT[:, 0:128], rhs=pg_sb[:, 0:512],
                    start=True, stop=True,
                )

        dummy_mms(0)

        # w2[e,c] = sum_d m2[d,e] w'[c,d]
        pw = ps_sm.tile([128, 256], f32, tag="sm")
        nc.tensor.matmul(pw, lhsT=m2_sb, rhs=wT, start=True, stop=True)
        nc.scalar.copy(out=w2_sb, in_=pw)
        dummy_mms(1)

        # ut[ci,c] = sum_e theta_w[e,ci] w2[e,c]   (= U[c,ci])
        for ci in range(2):
            pu = ps_sm.tile([128, 256], f32, tag="sm")
            nc.tensor.matmul(
                pu, lhsT=thw_sb[:, ci * 128 : (ci + 1) * 128], rhs=w2_sb,
                start=True, stop=True,
            )
            nc.scalar.copy(out=ut_sb[:, ci * 256 : (ci + 1) * 256], in_=pu)

        # b_z[c] = sum_e w2[e,c] theta_b[e] + b'[c]
        for cc in range(2):
            pb = ps_sm.tile([128, 1], f32, tag="sm")
            nc.tensor.matmul(
                pb, lhsT=w2_sb[:, cc * 128 : (cc + 1) * 128],
                rhs=thw_sb[:, 256:257], start=True, stop=True,
            )
            nc.scalar.activation(
                out=bz_sb[:, cc : cc + 1], in_=pb, func=AF.Identity,
                bias=b_out[cc], scale=1.0,
            )
        dummy_mms(2)

        # z[c,n] = sum_ci U[c,ci] x[ci,n] + b_z[c] + x[c,n]
        ndma = 0
        for j in range(NR):
            for cc in range(2):
                jsl = slice(j * 1024, (j + 1) * 1024)
                pz = ps_mm.tile([128, 1024], f32, tag="mm")
                for f in range(2):
                    fsl = slice(j * 1024 + f * 512, j * 1024 + (f + 1) * 512)
                    psl = slice(f * 512, (f + 1) * 512)
                    nc.tensor.matmul(
                        pz[:, psl],
                        lhsT=ut_sb[:, cc * 128 : (cc + 1) * 128],
                        rhs=X0[:, fsl], start=True, stop=False,
                    )
                    nc.tensor.matmul(
                        pz[:, psl],
                        lhsT=ut_sb[:, 256 + cc * 128 : 256 + (cc + 1) * 128],
                        rhs=X1[:, fsl], start=False, stop=True,
                    )
                xres = (X0 if cc == 0 else X1)[:, jsl]
                z_sb = zpool.tile([128, 1024], f32, tag="z_sb")
                nc.vector.scalar_tensor_tensor(
                    out=z_sb, in0=pz, scalar=bz_sb[:, cc : cc + 1],
                    in1=xres, op0=ALU.add, op1=ALU.add,
                )
                eng = nc.sync if ndma % 2 == 0 else nc.scalar
                ndma += 1
                eng.dma_start(
                    out=out[cc * 128 : (cc + 1) * 128, jsl], in_=z_sb,
                )

    nc.compile()
    return nc


def _get_nc():
    global _NC
    if _NC is None:
        _NC = _build_nc()
    return _NC


# test.py reads this after a traced run to get exec_time_ns
last_results = None


def _prep_inputs(inputs):
    import ml_dtypes

    bf16 = ml_dtypes.bfloat16

    x = np.asarray(inputs["x"], dtype=np.float32)
    theta_w = np.asarray(inputs["theta_w"], np.float32)
    theta_b = np.asarray(inputs["theta_b"], np.float32)
    phi_w = np.asarray(inputs["phi_w"], np.float32)
    phi_b = np.asarray(inputs["phi_b"], np.float32)
    g_w = np.asarray(inputs["g_w"], np.float32)
    g_b = np.asarray(inputs["g_b"], np.float32)
    w_w = np.asarray(inputs["w_w"], np.float32)
    w_b = np.asarray(inputs["w_b"], np.float32)
    bn_gamma = np.asarray(inputs["bn_gamma"], np.float32)
    bn_beta = np.asarray(inputs["bn_beta"], np.float32)
    bn_mean = np.asarray(inputs["bn_mean"], np.float32)
    bn_var = np.asarray(inputs["bn_var"], np.float32)

    inv = bn_gamma / np.sqrt(bn_var + BN_EPS)
    b_out = (w_b - bn_mean) * inv + bn_beta                   # (C,)

    aux = np.zeros((128, 260), np.float32)
    aux[:, 0] = b_out[:128]
    aux[:, 1] = b_out[128:]
    aux[:, 4:260] = np.concatenate([phi_b / N, g_b])[None, :]

    pgw = np.concatenate([phi_w.T / N, g_w.T], axis=1)        # (C, 2D)
    pgw_pk = np.concatenate([pgw[0:128], pgw[128:256]], axis=1)  # (128, 512)
    thw = np.zeros((D, 260), np.float32)
    thw[:, :256] = theta_w
    thw[:, 256] = theta_b
    wwt = (w_w * inv[:, None]).T                              # (D, C)

    wpk_u8 = np.concatenate(
        [
            aux.view(np.uint8),                               # 1040 B
            np.ascontiguousarray(pgw_pk).astype(bf16).view(np.uint8),  # 1024 B
            np.ascontiguousarray(thw).astype(bf16).view(np.uint8),     # 520 B
            np.ascontiguousarray(wwt).astype(bf16).view(np.uint8),     # 512 B
        ],
        axis=1,
    )
    assert wpk_u8.shape == (128, 3096), wpk_u8.shape
    wpk = np.ascontiguousarray(wpk_u8).view(np.float32)       # (128, 774)

    xf = x.reshape(B, C, N).astype(bf16)
    return xf, {"wpk": wpk}


def kernel(**inputs):
    from concourse.bass_utils import run_bass_kernel_spmd

    global last_results

    xf, shared = _prep_inputs(inputs)
    in_maps = [dict(shared, x=np.ascontiguousarray(xf[b])) for b in range(B)]

    nc = _get_nc()
    res = run_bass_kernel_spmd(nc, in_maps, list(range(N_CORES)))
    last_results = res

    z = np.stack([res.results[b]["out"] for b in range(B)])
    return z.reshape(B, C, HH, WW).astype(np.float32)
